# revision 1
# baseline (speedup 1.0000x reference)
"""VQ codebook reconstruction kernel for Trainium2 (8 NeuronCores, SPMD).

Reference computation (per pixel feature vector f in R^C):
    weights = (codebook @ f) / ||codebook_rows||^2      # [N]
    recon   = codebook.T @ weights                      # [C]

This collapses to a single fixed matrix applied per pixel:
    recon = M @ f,   M = codebook.T @ diag(1/||c_n||^2) @ codebook   # [C, C]

M is tiny ([256,256]) and is formed on the host in float64; the device
kernel applies M to all B*H*W = 131072 pixel vectors, sharded
data-parallel over (B, H) across 8 cores. Matmuls use float32r
(fp32 with 11-bit mantissa) which streams at full PE rate (1
cycle/row) for moving dim >= 256, unlike plain fp32 (4 cycles/row).
The output is written as fp16 (RNE, exact host upcast to fp32) to
halve write traffic; total scale-relative error ~4.8e-4. The kernel
is read-bandwidth-bound in the front half (16.9 MB/core at ~420 GB/s)
and matmul+copy-pipeline-bound in the back half, at ~81 us measured,
plus the fixed ~7 us NEFF preamble and ~10 us exit barrier.
"""

import numpy as np

B, C, H, W = 4, 256, 128, 256
N_CORES = 8
SPLIT_H = 2            # 8 shards = B(4) x H-halves(2)
SH = H // SPLIT_H      # 64 rows of H per shard
P_SHARD = SH * W       # 16384 pixels per core
TILE_N = 512
N_TILES = P_SHARD // TILE_N  # 32

_NC_CACHE = {}


def _build_nc():
    if "nc" in _NC_CACHE:
        return _NC_CACHE["nc"]

    import concourse.bass as bass
    import concourse.tile as tile
    from concourse import bacc, mybir

    f32 = mybir.dt.float32
    f16 = mybir.dt.float16
    f32r = mybir.dt.float32r

    nc = bacc.Bacc()
    feat = nc.dram_tensor("feat", [C, P_SHARD], f32r, kind="ExternalInput")
    mmat = nc.dram_tensor("mmat", [C, C], f32r, kind="ExternalInput")
    # fp16 output halves write traffic; host upcasts to fp32 (exact).
    # Output magnitudes are O(10) — far inside fp16 range; quantization
    # adds ~4.9e-4 scale-relative error on top of f32r's 2.4e-4.
    out = nc.dram_tensor("out", [C, P_SHARD], f16, kind="ExternalOutput")

    SLAB = 2048
    N_SLABS = P_SHARD // SLAB          # 8
    SUB = SLAB // TILE_N               # 4 matmul subtiles per slab

    # feat rows are (kb*128 + p); view as [p, kb, n] so one DMA per slab
    # pulls both K-halves.
    feat3 = feat.rearrange("(a k) n -> k a n", a=2)

    with tile.TileContext(nc) as tc:
        with (
            tc.tile_pool(name="mpool", bufs=1) as mpool,
            tc.tile_pool(name="rhs", bufs=8) as rhs_pool,
            tc.tile_pool(name="opool", bufs=3) as opool,
            tc.tile_pool(name="psum", bufs=4, space="PSUM") as psum_pool,
        ):
            # M as two [128, 256] K-halves; lhsT block for (kb, mb) is
            # m_tiles[kb][:, mb*128:(mb+1)*128] (M is symmetric so lhsT = M).
            m_tiles = []
            for kb in range(2):
                mt = mpool.tile([128, C], f32r, tag=f"m{kb}")
                nc.gpsimd.dma_start(mt[:], mmat[kb * 128:(kb + 1) * 128, :])
                m_tiles.append(mt)

            for j in range(N_SLABS):
                rt = rhs_pool.tile([128, 2, SLAB], f32r, tag="r")
                eng = nc.sync if (j % 2 == 0) else nc.scalar
                eng.dma_start(rt[:], feat3[:, :, bass.ts(j, SLAB)])
                ot = [
                    opool.tile([128, SLAB], f16, tag=f"o{mb}", name=f"ot{mb}")
                    for mb in range(2)
                ]
                for n in range(SUB):
                    for mb in range(2):
                        ps = psum_pool.tile([128, TILE_N], f32, tag=f"ps{mb}")
                        for kb in range(2):
                            nc.tensor.matmul(
                                ps[:],
                                m_tiles[kb][:, mb * 128:(mb + 1) * 128],
                                rt[:, kb, bass.ts(n, TILE_N)],
                                start=(kb == 0),
                                stop=(kb == 1),
                            )
                        nc.vector.tensor_copy(ot[mb][:, bass.ts(n, TILE_N)], ps[:])
                for mb in range(2):
                    nc.gpsimd.dma_start(
                        out[mb * 128:(mb + 1) * 128, bass.ts(j, SLAB)], ot[mb][:]
                    )

    nc.compile()
    _NC_CACHE["nc"] = nc
    return nc


def _host_prep(feature, codebook):
    cb = codebook.astype(np.float64)
    norm = np.sum(cb * cb, axis=1)
    m = ((cb / norm[:, None]).T @ cb).astype(np.float32)

    in_maps = []
    for i in range(N_CORES):
        b, hs = i // SPLIT_H, (i % SPLIT_H) * SH
        shard = np.ascontiguousarray(
            feature[b, :, hs:hs + SH, :].reshape(C, P_SHARD)
        )
        in_maps.append({"feat": shard, "mmat": m})
    return in_maps


def _gather(results):
    out = np.empty((B, C, H, W), dtype=np.float32)
    for i in range(N_CORES):
        b, hs = i // SPLIT_H, (i % SPLIT_H) * SH
        out[b, :, hs:hs + SH, :] = results[i]["out"].reshape(C, SH, W).astype(np.float32)
    return out


def run(feature, codebook, **spmd_kwargs):
    from concourse.bass_utils import run_bass_kernel_spmd

    nc = _build_nc()
    in_maps = _host_prep(np.asarray(feature), np.asarray(codebook))
    res = run_bass_kernel_spmd(nc, in_maps, list(range(N_CORES)), **spmd_kwargs)
    return _gather(res.results), res


def kernel(feature, codebook):
    out, _ = run(feature, codebook)
    return out



# revision 4
# speedup vs baseline: 1.3033x; 1.3033x over previous
"""VQ codebook reconstruction kernel for Trainium2 (8 NeuronCores, SPMD).

Reference computation (per pixel feature vector f in R^C):
    weights = (codebook @ f) / ||codebook_rows||^2      # [N]
    recon   = codebook.T @ weights                      # [C]

This collapses to a single fixed matrix applied per pixel:
    recon = M @ f,   M = codebook.T @ diag(1/||c_n||^2) @ codebook   # [C, C]

M is tiny ([256,256], symmetric), formed on the host in float64. The device
kernel applies M to all B*H*W = 131072 pixel vectors, sharded data-parallel
over (B, H-halves) across 8 cores.

v2 design (vs the 85us fp32r baseline):
  - The whole data path is fp16: the host pre-quantizes the feature shard
    and M to fp16, halving DMA-in traffic (16.8 -> 8.4 MB/core). Total HBM
    traffic is 16.8 MB/core at the ~420 GB/s per-core DMA ceiling -> ~40us
    floor. PE streams fp16 at 1 row/cycle (same as f32r) = 27us warm, and
    fp16 weights enable Fast Weight Load (fp32 cannot).
  - Host pre-shuffles the shard into [128, slab, kb, n] so every slab DMA
    is one 4KB-contiguous-per-partition 512KB transfer.
  - kb-outer matmul ordering: 4 weight switches per slab instead of 16.
  - PSUM->SBUF casts split between vector (mb=0) and scalar (mb=1).
  - 4 DMA queues: input on sync+gpsimd, output issued by vector+scalar.
  - Output written as fp16 (exact host upcast); quantization adds ~5e-4
    scale-relative error, far under the 2e-2 gate.
"""

import numpy as np

B, C, H, W = 4, 256, 128, 256
N_CORES = 8
SPLIT_H = 2            # 8 shards = B(4) x H-halves(2)
SH = H // SPLIT_H      # 64 rows of H per shard
P_SHARD = SH * W       # 16384 pixels per core
SLAB = 1024
N_SLABS = P_SHARD // SLAB    # 16
TILE_N = 512
SUB = SLAB // TILE_N         # 2 matmul subtiles per slab per mb

_NC_CACHE = {}


def _build_nc():
    if "nc" in _NC_CACHE:
        return _NC_CACHE["nc"]

    import concourse.bass as bass
    import concourse.tile as tile
    from concourse import bacc, mybir

    f32 = mybir.dt.float32
    f16 = mybir.dt.float16

    nc = bacc.Bacc()
    # feat[p, j, a, n] = f[a*128+p, j*SLAB+n]  (host pre-shuffled fp16)
    feat = nc.dram_tensor("feat", [128, N_SLABS, 2, SLAB], f16, kind="ExternalInput")
    # mmat[p, a, c] = M[a*128+p, c]
    mmat = nc.dram_tensor("mmat", [128, 2, C], f16, kind="ExternalInput")
    # out[p, j, mb, n] = recon[mb*128+p, j*SLAB+n]
    out = nc.dram_tensor("out", [128, N_SLABS, 2, SLAB], f16, kind="ExternalOutput")

    with tile.TileContext(nc) as tc:
        with (
            tc.tile_pool(name="mpool", bufs=1) as mpool,
            tc.tile_pool(name="rhs", bufs=8) as rhs_pool,
            tc.tile_pool(name="ov", bufs=4) as ov_pool,
            tc.tile_pool(name="os", bufs=4) as os_pool,
            tc.tile_pool(name="psum", bufs=2, space="PSUM") as psum_pool,
        ):
            mt = mpool.tile([128, 2, C], f16, tag="m")
            nc.scalar.dma_start(mt[:], mmat[:, :, :])

            for j in range(N_SLABS):
                rt = rhs_pool.tile([128, 2, SLAB], f16, tag="r", name=f"rt{j}")
                eng = nc.sync if (j % 2 == 0) else nc.gpsimd
                eng.dma_start(rt[:], feat[:, j, :, :])

                ps = {}
                for mb in range(2):
                    for n in range(SUB):
                        ps[(mb, n)] = psum_pool.tile(
                            [128, TILE_N], f32, tag=f"ps{mb}{n}", name=f"ps{mb}{n}"
                        )
                # kb-outer: one weight switch per (kb, mb) = 4 per slab.
                for kb in range(2):
                    for mb in range(2):
                        for n in range(SUB):
                            nc.tensor.matmul(
                                ps[(mb, n)][:],
                                mt[:, kb, mb * 128:(mb + 1) * 128],
                                rt[:, kb, bass.ts(n, TILE_N)],
                                start=(kb == 0),
                                stop=(kb == 1),
                            )
                otv = ov_pool.tile([128, SLAB], f16, tag="ov", name=f"otv{j}")
                ots = os_pool.tile([128, SLAB], f16, tag="os", name=f"ots{j}")
                for n in range(SUB):
                    nc.vector.tensor_copy(otv[:, bass.ts(n, TILE_N)], ps[(0, n)][:])
                    nc.scalar.copy(ots[:, bass.ts(n, TILE_N)], ps[(1, n)][:])
                nc.scalar.dma_start(out[:, j, 0, :], otv[:])
                nc.scalar.dma_start(out[:, j, 1, :], ots[:])

    nc.compile()
    _NC_CACHE["nc"] = nc
    return nc


def _host_prep(feature, codebook):
    cb = codebook.astype(np.float64)
    norm = np.sum(cb * cb, axis=1)
    m = ((cb / norm[:, None]).T @ cb).astype(np.float32)
    # m3[p, a, c] = M[a*128+p, c]
    m3 = np.ascontiguousarray(
        m.reshape(2, 128, C).transpose(1, 0, 2).astype(np.float16)
    )

    in_maps = []
    for i in range(N_CORES):
        b, hs = i // SPLIT_H, (i % SPLIT_H) * SH
        shard = feature[b, :, hs:hs + SH, :].reshape(C, P_SHARD)
        # f4[p, j, a, n] = shard[a*128+p, j*SLAB+n]
        f4 = np.ascontiguousarray(
            shard.reshape(2, 128, N_SLABS, SLAB)
            .transpose(1, 2, 0, 3)
            .astype(np.float16)
        )
        in_maps.append({"feat": f4, "mmat": m3})
    return in_maps


def _gather(results):
    out = np.empty((B, C, H, W), dtype=np.float32)
    for i in range(N_CORES):
        b, hs = i // SPLIT_H, (i % SPLIT_H) * SH
        o = results[i]["out"].astype(np.float32)   # [128, N_SLABS, 2, SLAB]
        shard = o.transpose(2, 0, 1, 3).reshape(C, SH, W)
        out[b, :, hs:hs + SH, :] = shard
    return out


def run(feature, codebook, **spmd_kwargs):
    from concourse.bass_utils import run_bass_kernel_spmd

    nc = _build_nc()
    in_maps = _host_prep(np.asarray(feature), np.asarray(codebook))
    res = run_bass_kernel_spmd(nc, in_maps, list(range(N_CORES)), **spmd_kwargs)
    return _gather(res.results), res


def kernel(feature, codebook):
    out, _ = run(feature, codebook)
    return out


# revision 5
# speedup vs baseline: 1.3738x; 1.0541x over previous
"""VQ codebook reconstruction kernel for Trainium2 (8 NeuronCores, SPMD).

Reference computation (per pixel feature vector f in R^C):
    weights = (codebook @ f) / ||codebook_rows||^2      # [N]
    recon   = codebook.T @ weights                      # [C]

This collapses to a single fixed matrix applied per pixel:
    recon = M @ f,   M = codebook.T @ diag(1/||c_n||^2) @ codebook   # [C, C]

M is tiny ([256,256], symmetric), formed on the host in float64. The device
kernel applies M to all B*H*W = 131072 pixel vectors, sharded data-parallel
over (B, H-halves) across 8 cores.

v3 design (85us fp32r baseline -> 65us v2 -> this):
  - Whole data path fp16 (host pre-quantizes): 16.8 MB/core total HBM
    traffic at the ~420 GB/s per-core DMA ceiling -> ~40us floor. fp16
    streams the PE at 1 row/cycle (like f32r) and enables Fast Weight Load.
  - 8 slabs of 2048 cols; each slab's input is split by K-half across the
    two input queues (sync/gpsimd) so queue service order matches compute
    order and the first matmul's prefill is halved.
  - One combined [128,2,2048] output DMA per slab, alternating sync/gpsimd,
    issued AFTER the next-next input so inputs never queue behind outputs.
    Scalar only casts + loads M (v2 lost ~20us to 33 dma_starts on scalar).
  - PSUM->SBUF casts split: vector does mb=0, scalar does mb=1.
  - PE warm-up: memset a dummy tile, run 8 throwaway matmuls during the
    input prefill so the HAM clock gate is at 2.4 GHz when real data lands.
"""

import numpy as np

B, C, H, W = 4, 256, 128, 256
N_CORES = 8
SPLIT_H = 2            # 8 shards = B(4) x H-halves(2)
SH = H // SPLIT_H      # 64 rows of H per shard
P_SHARD = SH * W       # 16384 pixels per core
SLAB = 2048
N_SLABS = P_SHARD // SLAB    # 8
TILE_N = 512
HALF = 1024                  # compute granularity inside a slab

_NC_CACHE = {}


def _build_nc():
    if "nc" in _NC_CACHE:
        return _NC_CACHE["nc"]

    import concourse.bass as bass
    import concourse.tile as tile
    from concourse import bacc, mybir

    f32 = mybir.dt.float32
    f16 = mybir.dt.float16

    nc = bacc.Bacc()
    # feat[p, j, a, n] = f[a*128+p, j*SLAB+n]  (host pre-shuffled fp16)
    feat = nc.dram_tensor("feat", [128, N_SLABS, 2, SLAB], f16, kind="ExternalInput")
    # mmat[p, a, c] = M[a*128+p, c]
    mmat = nc.dram_tensor("mmat", [128, 2, C], f16, kind="ExternalInput")
    # out[p, j, mb, n] = recon[mb*128+p, j*SLAB+n]
    out = nc.dram_tensor("out", [128, N_SLABS, 2, SLAB], f16, kind="ExternalOutput")

    in_engs = [None, None]

    with tile.TileContext(nc) as tc:
        in_engs[0] = nc.sync
        in_engs[1] = nc.gpsimd
        with (
            tc.tile_pool(name="mpool", bufs=1) as mpool,
            tc.tile_pool(name="warm", bufs=1) as warm_pool,
            tc.tile_pool(name="rhs", bufs=4) as rhs_pool,
            tc.tile_pool(name="opool", bufs=3) as opool,
            tc.tile_pool(name="psum", bufs=2, space="PSUM") as psum_pool,
        ):
            mt = mpool.tile([128, 2, C], f16, tag="m")
            nc.sync.dma_start(mt[:], mmat[:, :, :])

            rts = [rhs_pool.tile([128, 2, SLAB], f16, tag="r", name=f"rt{j}")
                   for j in range(N_SLABS)]

            def issue_in(j):
                # K-half a goes to queue a: service order == consume order.
                for a in range(2):
                    in_engs[a].dma_start(rts[j][:, a, :], feat[:, j, a, :])

            issue_in(0)
            issue_in(1)

            # PE warm-up: 8 self-contained matmuls on a memset tile keep the
            # PE busy through the HAM activity window during input prefill.
            wt = warm_pool.tile([128, TILE_N], f16, tag="w")
            nc.vector.memset(wt[:], 1.0)
            for i in range(8):
                pw = psum_pool.tile([128, TILE_N], f32, tag="ps00", name=f"pw{i}")
                nc.tensor.matmul(pw[:], wt[:, 0:128], wt[:], start=True, stop=True)

            for j in range(N_SLABS):
                if j + 2 < N_SLABS:
                    issue_in(j + 2)
                rt = rts[j]
                ot = opool.tile([128, 2, SLAB], f16, tag="o", name=f"ot{j}")
                for h in range(2):
                    ps = {}
                    for mb in range(2):
                        for n in range(2):
                            ps[(mb, n)] = psum_pool.tile(
                                [128, TILE_N], f32, tag=f"ps{mb}{n}", name=f"ps{mb}{n}"
                            )
                    # kb-outer: weight switches only when (kb, mb) changes.
                    for kb in range(2):
                        for mb in range(2):
                            for n in range(2):
                                nc.tensor.matmul(
                                    ps[(mb, n)][:],
                                    mt[:, kb, mb * 128:(mb + 1) * 128],
                                    rt[:, kb, bass.ts(2 * h + n, TILE_N)],
                                    start=(kb == 0),
                                    stop=(kb == 1),
                                )
                    for n in range(2):
                        nc.vector.tensor_copy(
                            ot[:, 0, bass.ts(2 * h + n, TILE_N)], ps[(0, n)][:]
                        )
                        nc.scalar.copy(
                            ot[:, 1, bass.ts(2 * h + n, TILE_N)], ps[(1, n)][:]
                        )
                in_engs[j % 2].dma_start(out[:, j, :, :], ot[:])

    nc.compile()
    _NC_CACHE["nc"] = nc
    return nc


def _host_prep(feature, codebook):
    cb = codebook.astype(np.float64)
    norm = np.sum(cb * cb, axis=1)
    m = ((cb / norm[:, None]).T @ cb).astype(np.float32)
    # m3[p, a, c] = M[a*128+p, c]
    m3 = np.ascontiguousarray(
        m.reshape(2, 128, C).transpose(1, 0, 2).astype(np.float16)
    )

    in_maps = []
    for i in range(N_CORES):
        b, hs = i // SPLIT_H, (i % SPLIT_H) * SH
        shard = feature[b, :, hs:hs + SH, :].reshape(C, P_SHARD)
        # f4[p, j, a, n] = shard[a*128+p, j*SLAB+n]
        f4 = np.ascontiguousarray(
            shard.reshape(2, 128, N_SLABS, SLAB)
            .transpose(1, 2, 0, 3)
            .astype(np.float16)
        )
        in_maps.append({"feat": f4, "mmat": m3})
    return in_maps


def _gather(results):
    out = np.empty((B, C, H, W), dtype=np.float32)
    for i in range(N_CORES):
        b, hs = i // SPLIT_H, (i % SPLIT_H) * SH
        o = results[i]["out"].astype(np.float32)   # [128, N_SLABS, 2, SLAB]
        shard = o.transpose(2, 0, 1, 3).reshape(C, SH, W)
        out[b, :, hs:hs + SH, :] = shard
    return out


def run(feature, codebook, **spmd_kwargs):
    from concourse.bass_utils import run_bass_kernel_spmd

    nc = _build_nc()
    in_maps = _host_prep(np.asarray(feature), np.asarray(codebook))
    res = run_bass_kernel_spmd(nc, in_maps, list(range(N_CORES)), **spmd_kwargs)
    return _gather(res.results), res


def kernel(feature, codebook):
    out, _ = run(feature, codebook)
    return out
